# revision 20
# baseline (speedup 1.0000x reference)
"""CrossSeqAttentionLayer on 8 TRN2 NeuronCores.

Sharding: query-row split (no collectives). Core c handles batch c//2,
query rows (c%2)*1024 .. +1024, all 16 heads. Each core computes K/V for
its batch's full 2048 keys, attention for its 1024 query rows, the output
projection and layernorm for those rows, and writes [1024, 1024] f32.

Device-side structure (v2 — engine-balanced, PE-dense):
  - S^T per head-pair via ROW-TILED concurrent matmuls (K=64 in rows 0-63
    and 64-127 -> 2 MMs per 512-cycle slot).
  - One exp per [128 keys, 1024] PSUM tile (hi0|hi1) on ACT, then one DVE
    mul with rfac broadcast over the two head halves.
  - PV col-tiled: head A -> PSUM partitions 0-63, head B -> 64-127 of the
    same bank (concurrent M=64 matmuls). Softmax denominators via M=1
    ones-matmuls into strips of one shared Z bank (also col-concurrent).
  - K/Q projections of the next pair are interleaved into the attention
    stream as PE filler; V projection runs up front.
  - Out-projection contracts head PAIRS (K=128) with stationary reuse.
  - PSUM: 2x sps[128,1024] (4 banks) + 2x pv[128,512] + 1x Z + 1x proj = 8.
"""

import numpy as np
import ml_dtypes

import concourse.bass as bass
import concourse.mybir as mybir
import concourse.tile as tile
from concourse import bacc
from concourse.bass_utils import run_bass_kernel_spmd

BF16 = mybir.dt.bfloat16
F32 = mybir.dt.float32
NPBF16 = ml_dtypes.bfloat16

B, L, D = 4, 2048, 1024
H, DH = 16, 64
NCORE = 8
LQ = L // 2            # query rows per core
SCALE = DH ** -0.5
EPS_R = 1e-8
EPS_LN = 1e-5
NP = 8                 # head pairs
KB = L // 128          # key blocks of 128
DT = D // 128          # contraction tiles of 128


def _emit(tc, aps, skip_affine):
    nc = tc.nc
    (tokT, tokTq, wqp, wkp, wv, wph2, rfT, resid, gamma, lnb, out) = aps

    import contextlib
    with contextlib.ExitStack() as ctx:
        # SBUF pools
        big = ctx.enter_context(tc.tile_pool(name="big", bufs=64))   # resident [128,1024]-ish bf16
        ktp = ctx.enter_context(tc.tile_pool(name="ktp", bufs=2))    # [128,2048] kt per pair
        qtp = ctx.enter_context(tc.tile_pool(name="qtp", bufs=2))    # [128,1024] qt per pair
        wqk = ctx.enter_context(tc.tile_pool(name="wqk", bufs=32))   # [128,128] weight blocks
        expp = ctx.enter_context(tc.tile_pool(name="expp", bufs=3))  # e tiles [128,1024] bf16
        aop = ctx.enter_context(tc.tile_pool(name="aop", bufs=8))    # aoT2 [128,1024] bf16
        bcp = ctx.enter_context(tc.tile_pool(name="bcp", bufs=2))    # broadcast [128,1024] bf16
        zip_ = ctx.enter_context(tc.tile_pool(name="zip", bufs=4))   # zi2 [1,1024] bf16
        zsp = ctx.enter_context(tc.tile_pool(name="zsp", bufs=2))    # zsb [128,512] bf16
        smp = ctx.enter_context(tc.tile_pool(name="smp", bufs=8))   # small tiles
        xp = ctx.enter_context(tc.tile_pool(name="xp", bufs=2))      # x [128,1024] f32
        gbp = ctx.enter_context(tc.tile_pool(name="gbp", bufs=2))
        # PSUM pools — exactly 8 banks total
        sps_pool = ctx.enter_context(
            tc.tile_pool(name="sps", bufs=2, space=bass.MemorySpace.PSUM))   # [128,1024] -> 2 banks each
        pvz = ctx.enter_context(
            tc.tile_pool(name="pvz", bufs=3, space=bass.MemorySpace.PSUM))   # [128,512] pv0, pv1, zb
        prj = ctx.enter_context(
            tc.tile_pool(name="prj", bufs=1, space=bass.MemorySpace.PSUM))   # [128,512] proj chunks

        # ---- resident loads ----
        tokT_sb = []  # 16 tiles [128, 1024]: index t*2+half
        for t in range(DT):
            for half in range(2):
                s = big.tile([128, 1024], BF16, tag="big")
                nc.sync.dma_start(out=s, in_=tokT[t * 128:(t + 1) * 128,
                                               half * 1024:(half + 1) * 1024])
                tokT_sb.append(s)
        wv_sb = []
        for t in range(DT):
            s = big.tile([128, 1024], BF16, tag="big")
            nc.sync.dma_start(out=s, in_=wv[t * 128:(t + 1) * 128, :])
            wv_sb.append(s)
        tokTq_sb = []
        for t in range(DT):
            s = big.tile([128, 1024], BF16, tag="big")
            nc.sync.dma_start(out=s, in_=tokTq[t * 128:(t + 1) * 128, :])
            tokTq_sb.append(s)

        ones_t = smp.tile([128, 1], BF16, tag="ones")
        nc.vector.memset(ones_t, 1.0)
        eps_t = smp.tile([128, 1], F32, tag="eps")
        nc.vector.memset(eps_t, EPS_LN)
        if not skip_affine:
            gamma_b = gbp.tile([128, 1024], BF16)
            nc.gpsimd.dma_start(out=gamma_b, in_=gamma.partition_broadcast(128))
            lnb_b = gbp.tile([128, 1024], BF16)
            nc.gpsimd.dma_start(out=lnb_b, in_=lnb.partition_broadcast(128))

        # ---- V projection: va[kb] = V rows for 128 keys x all 16 heads ----
        va_sb = []
        for kb in range(KB):
            ps = sps_pool.tile([128, 1024], F32, tag="sps", name="psv")
            for vc in range(2):
                for t in range(DT):
                    lhs = tokT_sb[t * 2 + kb // 8][:, (kb % 8) * 128:(kb % 8 + 1) * 128]
                    nc.tensor.matmul(ps[:, vc * 512:(vc + 1) * 512], lhs,
                                     wv_sb[t][:, vc * 512:(vc + 1) * 512],
                                     start=(t == 0), stop=(t == DT - 1))
            va = big.tile([128, 1024], BF16, tag="big")
            nc.vector.tensor_copy(va, ps)
            va_sb.append(va)

        # ---- weight-block DMA prefetch for K/Q of pair p ----
        wkb_sb = [None] * NP
        wqb_sb = [None] * NP

        rfT_sb = [None] * KB

        def load_rf():
            for kb in range(KB):
                s = big.tile([128, 1024], BF16, tag="big")
                nc.sync.dma_start(out=s, in_=rfT[kb * 128:(kb + 1) * 128, :])
                rfT_sb[kb] = s

        def prefetch_w(p):
            wkb_sb[p] = [wqk.tile([128, 128], BF16, tag="wqk", name="wkb") for _ in range(DT)]
            for t in range(DT):
                nc.sync.dma_start(out=wkb_sb[p][t], in_=wkp[p, t * 128:(t + 1) * 128, :])
            wqb_sb[p] = [wqk.tile([128, 128], BF16, tag="wqk", name="wqb") for _ in range(DT)]
            for t in range(DT):
                nc.sync.dma_start(out=wqb_sb[p][t], in_=wqp[p, t * 128:(t + 1) * 128, :])

        kt_sb = [None] * NP   # [128, 2048] per pair (2x64 dh rows x keys)
        qt_sb = [None] * NP   # [128, 1024]

        def emit_proj_chunk(p, ci, pool=None):
            """One K/Q projection chunk for pair p: 8 MMs + 1 drain.
            ci 0-3: K chunks (half, kc); ci 4-5: Q chunks (qc)."""
            if ci == 0:
                kt_sb[p] = ktp.tile([128, 2048], BF16, tag="kt", name="kt")
            if ci == 4:
                qt_sb[p] = qtp.tile([128, 1024], BF16, tag="qt", name="qt")
            psk = (pool or prj).tile([128, 512], F32, tag="pvz" if pool else "prj",
                                     name="psk")
            if ci < 4:
                half, kc = ci // 2, ci % 2
                for t in range(DT):
                    nc.tensor.matmul(psk, wkb_sb[p][t],
                                     tokT_sb[t * 2 + half][:, kc * 512:(kc + 1) * 512],
                                     start=(t == 0), stop=(t == DT - 1))
                col = half * 1024 + kc * 512
                nc.vector.tensor_copy(kt_sb[p][:, col:col + 512], psk)
            else:
                qc = ci - 4
                for t in range(DT):
                    nc.tensor.matmul(psk, wqb_sb[p][t],
                                     tokTq_sb[t][:, qc * 512:(qc + 1) * 512],
                                     start=(t == 0), stop=(t == DT - 1))
                nc.vector.tensor_copy(qt_sb[p][:, qc * 512:(qc + 1) * 512], psk)

        # ---- attention per pair ----
        aoT2 = [None] * NP    # [128, 1024] bf16: rows 0-63 head 2p, 64-127 head 2p+1
        zsb_sb = [None] * NP  # [128, 512] bf16: denominators at rows 0/32/64/96

        def drain_pair(p, pv, zb):
            """Fast pv/zb release at pair boundary: raw copies only."""
            aoT2[p] = aop.tile([128, 1024], BF16, tag="aoT2", name="aoT2")
            for qh in range(2):
                nc.scalar.copy(aoT2[p][:, qh * 512:(qh + 1) * 512], pv[qh])
            zsb_sb[p] = zsp.tile([128, 512], F32, tag="zsb", name="zsb")
            nc.vector.tensor_copy(zsb_sb[p], zb)

        def norm_step(p, qh):
            """Deferred normalization: scale aoT2[p] q-half by 1/Z (per head).
            zi2 = [1/Z(hi0) | 1/Z(hi1)] broadcast to all 128 partitions; each
            hi-half of aoT2 multiplies against its half at a MATCHING base
            partition (SBUF TT requires equal input base partitions)."""
            qsl = slice(qh * 512, (qh + 1) * 512)
            zi2 = zip_.tile([1, 1024], BF16, tag="zi2", name="zi2")
            with nc.allow_low_precision(reason="softmax denominators fit bf16"):
                for hi in range(2):
                    zrow = 32 * (2 * hi + qh)
                    nc.vector.reciprocal(zi2[:, hi * 512:(hi + 1) * 512],
                                         zsb_sb[p][zrow:zrow + 1, :])
            bc = bcp.tile([128, 1024], BF16, tag="bc", name="bc")
            nc.gpsimd.partition_broadcast(bc, zi2)
            for hi in range(2):
                hsl = slice(hi * 64, (hi + 1) * 64)
                nc.vector.tensor_mul(aoT2[p][hsl, qsl], aoT2[p][hsl, qsl],
                                     bc[hsl, hi * 512:(hi + 1) * 512])

        for p in range(NP):
            if p == 0:
                prefetch_w(0)
                prefetch_w(1)
                load_rf()
                for ci in range(6):
                    emit_proj_chunk(0, ci, pool=pvz)
            pv = [pvz.tile([128, 512], F32, tag="pvz", name="pv") for _ in range(2)]
            zb = pvz.tile([128, 512], F32, tag="pvz", name="zb")
            kt, qt = kt_sb[p], qt_sb[p]
            nci = 0  # next proj chunk to emit for pair p+1
            for it in range(32):
                kb, qh = it // 2, it % 2
                qsl = slice(qh * 512, (qh + 1) * 512)
                kcol = slice(kb * 128, (kb + 1) * 128)
                # S^T: row-tiled concurrent pair
                sps = sps_pool.tile([128, 1024], F32, tag="sps", name="sps")
                nc.tensor.matmul(sps[:, 0:512], kt[0:64, kcol], qt[0:64, qsl],
                                 start=True, stop=True)
                nc.tensor.matmul(sps[:, 512:1024], kt[64:128, kcol], qt[64:128, qsl],
                                 start=True, stop=True)
                # exp on ACT (PSUM -> SBUF bf16), then rfac muls on DVE (2x mode)
                e = expp.tile([128, 1024], BF16, tag="e", name="e")
                nc.scalar.activation(e, sps, mybir.ActivationFunctionType.Exp,
                                     scale=SCALE)
                rf = rfT_sb[kb][:, qsl]
                nc.vector.tensor_mul(e[:, 0:512], e[:, 0:512], rf)
                nc.vector.tensor_mul(e[:, 512:1024], e[:, 512:1024], rf)
                # PV col-tiled pair: A -> partitions 0-63, B -> 64-127
                nc.tensor.matmul(pv[qh][0:64, :], va_sb[kb][:, (2 * p) * 64:(2 * p) * 64 + 64],
                                 e[:, 0:512], start=(kb == 0), stop=(kb == KB - 1))
                nc.tensor.matmul(pv[qh][64:128, :], va_sb[kb][:, (2 * p + 1) * 64:(2 * p + 1) * 64 + 64],
                                 e[:, 512:1024], start=(kb == 0), stop=(kb == KB - 1))
                # Z: M=1 ones-matmuls, col strips qh and 2+qh
                nc.tensor.matmul(zb[32 * qh:32 * qh + 1, :], ones_t[:, 0:1],
                                 e[:, 0:512], start=(kb == 0), stop=(kb == KB - 1),
                                 tile_position=(0, 32 * qh))
                nc.tensor.matmul(zb[64 + 32 * qh:64 + 32 * qh + 1, :], ones_t[:, 0:1],
                                 e[:, 512:1024], start=(kb == 0), stop=(kb == KB - 1),
                                 tile_position=(0, 64 + 32 * qh))
                # interleave: next pair's K/Q proj chunks as PE filler
                if p + 1 < NP and it in (4, 9, 14, 19, 24, 29):
                    emit_proj_chunk(p + 1, nci)
                    nci += 1
                # deferred normalization of the previous pair, off critical path
                if p >= 1 and it == 6:
                    norm_step(p - 1, 0)
                if p >= 1 and it == 16:
                    norm_step(p - 1, 1)
                # deferred normalization of the previous pair, off critical path
            if p + 2 < NP:
                prefetch_w(p + 2)
            drain_pair(p, pv, zb)
        norm_step(NP - 1, 0)
        norm_step(NP - 1, 1)


        # ---- out projection + residual + layernorm ----
        wph_sb = []
        for hp in range(NP):
            s = big.tile([128, 1024], BF16, tag="big")
            nc.sync.dma_start(out=s, in_=wph2[hp])
            wph_sb.append(s)
        resid_sb = []
        for rb in range(8):
            s = big.tile([128, 1024], BF16, tag="big")
            nc.sync.dma_start(out=s, in_=resid[rb * 128:(rb + 1) * 128, :])
            resid_sb.append(s)

        for rb in range(8):
            rsl = slice(rb * 128, (rb + 1) * 128)
            psy = sps_pool.tile([128, 1024], F32, tag="sps", name="psy")
            for hp in range(NP):
                for nch in range(2):
                    nsl = slice(nch * 512, (nch + 1) * 512)
                    nc.tensor.matmul(psy[:, nsl], aoT2[hp][:, rsl],
                                     wph_sb[hp][:, nsl],
                                     start=(hp == 0), stop=(hp == NP - 1))
            x = xp.tile([128, 1024], F32, tag="x")
            nc.vector.tensor_add(x, psy, resid_sb[rb])
            st = smp.tile([128, 2, 6], F32, tag="st", name="st")
            for s2 in range(2):
                nc.vector.bn_stats(st[:, s2, :], x[:, s2 * 512:(s2 + 1) * 512])
            mv = smp.tile([128, 2], F32, tag="mv", name="mv")
            nc.vector.bn_aggr(mv, st)
            rstd = smp.tile([128, 1], F32, tag="rstd", name="rstd")
            nc.scalar.activation(rstd, mv[:, 1:2],
                                 mybir.ActivationFunctionType.Sqrt, bias=eps_t)
            nc.vector.reciprocal(rstd, rstd)
            nmr = smp.tile([128, 1], F32, tag="nmr", name="nmr")
            nc.vector.tensor_mul(nmr, mv[:, 0:1], rstd)
            nc.vector.tensor_scalar_mul(nmr, nmr, -1.0)
            nc.vector.tensor_scalar(x, x, nmr, rstd,
                                    mybir.AluOpType.add, mybir.AluOpType.mult)
            if not skip_affine:
                nc.vector.tensor_mul(x, x, gamma_b)
                nc.vector.tensor_add(x, x, lnb_b)
            nc.sync.dma_start(out=out[rsl, :], in_=x)


_CACHE = {}


def _build(skip_affine):
    key = bool(skip_affine)
    if key in _CACHE:
        return _CACHE[key]
    nc = bacc.Bacc("TRN2", target_bir_lowering=False, debug=False,
                   num_devices=NCORE)
    aps = (
        nc.dram_tensor("tokT", [D, L], BF16, kind="ExternalInput").ap(),
        nc.dram_tensor("tokTq", [D, LQ], BF16, kind="ExternalInput").ap(),
        nc.dram_tensor("wqp", [NP, D, 128], BF16, kind="ExternalInput").ap(),
        nc.dram_tensor("wkp", [NP, D, 128], BF16, kind="ExternalInput").ap(),
        nc.dram_tensor("wv", [D, D], BF16, kind="ExternalInput").ap(),
        nc.dram_tensor("wph2", [NP, 128, D], BF16, kind="ExternalInput").ap(),
        nc.dram_tensor("rfT", [L, LQ], BF16, kind="ExternalInput").ap(),
        nc.dram_tensor("resid", [LQ, D], BF16, kind="ExternalInput").ap(),
        nc.dram_tensor("gamma", [D], BF16, kind="ExternalInput").ap(),
        nc.dram_tensor("lnb", [D], BF16, kind="ExternalInput").ap(),
        nc.dram_tensor("out", [LQ, D], F32, kind="ExternalOutput").ap(),
    )
    with tile.TileContext(nc) as tc:
        _emit(tc, aps, skip_affine)
    nc.compile()
    _CACHE[key] = nc
    return nc


def kernel(tokens, R, Wq, Wk, Wv, Wp, beta, gamma, bias, _spmd_kwargs=None):
    tokens = np.asarray(tokens, dtype=np.float32)
    R = np.asarray(R, dtype=np.float32)
    skip_affine = bool(np.all(gamma == 1.0) and np.all(bias == 0.0))
    nc = _build(skip_affine)

    rfac = np.maximum(R, EPS_R).astype(np.float64) ** float(beta[0])
    rfacT = np.ascontiguousarray(rfac.T.astype(NPBF16))
    wqT = np.ascontiguousarray(Wq.T.astype(NPBF16))
    wkT = np.ascontiguousarray(Wk.T.astype(NPBF16))
    wvT = np.ascontiguousarray(Wv.T.astype(NPBF16))
    wpT = np.ascontiguousarray(Wp.T.astype(NPBF16))
    wqp = np.ascontiguousarray(wqT.reshape(D, NP, 128).transpose(1, 0, 2))
    wkp = np.ascontiguousarray(wkT.reshape(D, NP, 128).transpose(1, 0, 2))
    wph2 = np.ascontiguousarray(wpT.reshape(NP, 128, D))
    gamma_b = gamma.astype(NPBF16)
    lnb_b = bias.astype(NPBF16)

    in_maps = []
    for c in range(NCORE):
        b, q0 = c // 2, (c % 2) * LQ
        tokT = np.ascontiguousarray(tokens[b].T.astype(NPBF16))
        in_maps.append({
            "tokT": tokT,
            "tokTq": np.ascontiguousarray(tokT[:, q0:q0 + LQ]),
            "wqp": wqp, "wkp": wkp, "wv": wvT, "wph2": wph2,
            "rfT": np.ascontiguousarray(rfacT[:, q0:q0 + LQ]),
            "resid": np.ascontiguousarray(tokens[b, q0:q0 + LQ, :].astype(NPBF16)),
            "gamma": gamma_b, "lnb": lnb_b,
        })

    res = run_bass_kernel_spmd(nc, in_maps, core_ids=list(range(NCORE)),
                               **(_spmd_kwargs or {}))
    out = np.empty((B, L, D), dtype=np.float32)
    for c in range(NCORE):
        b, q0 = c // 2, (c % 2) * LQ
        out[b, q0:q0 + LQ, :] = res.results[c]["out"]
    if _spmd_kwargs:
        kernel._last_result = res
    return out


# revision 24
# speedup vs baseline: 1.1258x; 1.1258x over previous
"""CrossSeqAttentionLayer on 8 TRN2 NeuronCores.

Sharding: query-row split (no collectives). Core c handles batch c//2,
query rows (c%2)*1024 .. +1024, all 16 heads. Each core computes K/V for
its batch's full 2048 keys, attention for its 1024 query rows, the output
projection and layernorm for those rows, and writes [1024, 1024] f32.

Device-side structure (v2 — engine-balanced, PE-dense):
  - S^T per head-pair via ROW-TILED concurrent matmuls (K=64 in rows 0-63
    and 64-127 -> 2 MMs per 512-cycle slot).
  - One exp per [128 keys, 1024] PSUM tile (hi0|hi1) on ACT, then one DVE
    mul with rfac broadcast over the two head halves.
  - PV col-tiled: head A -> PSUM partitions 0-63, head B -> 64-127 of the
    same bank (concurrent M=64 matmuls). Softmax denominators via M=1
    ones-matmuls into strips of one shared Z bank (also col-concurrent).
  - K/Q projections of the next pair are interleaved into the attention
    stream as PE filler; V projection runs up front.
  - Out-projection contracts head PAIRS (K=128) with stationary reuse.
  - PSUM: 2x sps[128,1024] (4 banks) + 2x pv[128,512] + 1x Z + 1x proj = 8.
"""

import numpy as np
import ml_dtypes

import concourse.bass as bass
import concourse.mybir as mybir
import concourse.tile as tile
from concourse import bacc
from concourse.bass_utils import run_bass_kernel_spmd

BF16 = mybir.dt.bfloat16
F32 = mybir.dt.float32
NPBF16 = ml_dtypes.bfloat16

B, L, D = 4, 2048, 1024
H, DH = 16, 64
NCORE = 8
LQ = L // 2            # query rows per core
SCALE = DH ** -0.5
EPS_R = 1e-8
EPS_LN = 1e-5
NP = 8                 # head pairs
KB = L // 128          # key blocks of 128
DT = D // 128          # contraction tiles of 128


def _emit(tc, aps, skip_affine):
    nc = tc.nc
    (tokT, tokTq, wqp, wkp, wv, wph2, rfT, resid, gamma, lnb, out) = aps

    import contextlib
    with contextlib.ExitStack() as ctx:
        # SBUF pools
        big = ctx.enter_context(tc.tile_pool(name="big", bufs=64))   # resident [128,1024]-ish bf16
        ktp = ctx.enter_context(tc.tile_pool(name="ktp", bufs=2))    # [128,2048] kt per pair
        qtp = ctx.enter_context(tc.tile_pool(name="qtp", bufs=2))    # [128,1024] qt per pair
        wqk = ctx.enter_context(tc.tile_pool(name="wqk", bufs=32))   # [128,128] weight blocks
        expp = ctx.enter_context(tc.tile_pool(name="expp", bufs=3))  # e tiles [128,1024] bf16
        aop = ctx.enter_context(tc.tile_pool(name="aop", bufs=8))    # aoT2 [128,1024] bf16
        bcp = ctx.enter_context(tc.tile_pool(name="bcp", bufs=2))    # broadcast [128,1024] bf16
        zip_ = ctx.enter_context(tc.tile_pool(name="zip", bufs=4))   # zi2 [1,1024] bf16
        zsp = ctx.enter_context(tc.tile_pool(name="zsp", bufs=2))    # zsb [128,512] bf16
        smp = ctx.enter_context(tc.tile_pool(name="smp", bufs=8))   # small tiles
        xp = ctx.enter_context(tc.tile_pool(name="xp", bufs=2))      # x [128,1024] f32
        gbp = ctx.enter_context(tc.tile_pool(name="gbp", bufs=2))
        # PSUM pools — exactly 8 banks total
        sps_pool = ctx.enter_context(
            tc.tile_pool(name="sps", bufs=2, space=bass.MemorySpace.PSUM))   # [128,1024] -> 2 banks each
        pvz = ctx.enter_context(
            tc.tile_pool(name="pvz", bufs=3, space=bass.MemorySpace.PSUM))   # [128,512] pv0, pv1, zb
        prj = ctx.enter_context(
            tc.tile_pool(name="prj", bufs=1, space=bass.MemorySpace.PSUM))   # [128,512] proj chunks

        # ---- resident loads ----
        tokT_sb = []  # 16 tiles [128, 1024]: index t*2+half
        for t in range(DT):
            for half in range(2):
                s = big.tile([128, 1024], BF16, tag="big")
                nc.sync.dma_start(out=s, in_=tokT[t * 128:(t + 1) * 128,
                                               half * 1024:(half + 1) * 1024])
                tokT_sb.append(s)
        wv_sb = []
        for t in range(DT):
            s = big.tile([128, 1024], BF16, tag="big")
            nc.sync.dma_start(out=s, in_=wv[t * 128:(t + 1) * 128, :])
            wv_sb.append(s)
        tokTq_sb = []
        for t in range(DT):
            s = big.tile([128, 1024], BF16, tag="big")
            nc.sync.dma_start(out=s, in_=tokTq[t * 128:(t + 1) * 128, :])
            tokTq_sb.append(s)

        ones_t = smp.tile([128, 1], BF16, tag="ones")
        nc.vector.memset(ones_t, 1.0)
        eps_t = smp.tile([128, 1], F32, tag="eps")
        nc.vector.memset(eps_t, EPS_LN)
        if not skip_affine:
            gamma_b = gbp.tile([128, 1024], BF16)
            nc.gpsimd.dma_start(out=gamma_b, in_=gamma.partition_broadcast(128))
            lnb_b = gbp.tile([128, 1024], BF16)
            nc.gpsimd.dma_start(out=lnb_b, in_=lnb.partition_broadcast(128))

        # ---- V projection: va[kb] = V rows for 128 keys x all 16 heads ----
        va_sb = []
        for kb in range(KB):
            ps = sps_pool.tile([128, 1024], F32, tag="sps", name="psv")
            for vc in range(2):
                for t in range(DT):
                    lhs = tokT_sb[t * 2 + kb // 8][:, (kb % 8) * 128:(kb % 8 + 1) * 128]
                    nc.tensor.matmul(ps[:, vc * 512:(vc + 1) * 512], lhs,
                                     wv_sb[t][:, vc * 512:(vc + 1) * 512],
                                     start=(t == 0), stop=(t == DT - 1))
            va = big.tile([128, 1024], BF16, tag="big")
            nc.vector.tensor_copy(va, ps)
            va_sb.append(va)

        # ---- weight-block DMA prefetch for K/Q of pair p ----
        wkb_sb = [None] * NP
        wqb_sb = [None] * NP

        rfT_sb = [None] * KB

        def load_rf():
            for kb in range(KB):
                s = big.tile([128, 1024], BF16, tag="big")
                nc.sync.dma_start(out=s, in_=rfT[kb * 128:(kb + 1) * 128, :])
                rfT_sb[kb] = s

        def prefetch_w(p):
            wkb_sb[p] = [wqk.tile([128, 128], BF16, tag="wqk", name="wkb") for _ in range(DT)]
            for t in range(DT):
                nc.sync.dma_start(out=wkb_sb[p][t], in_=wkp[p, t * 128:(t + 1) * 128, :])
            wqb_sb[p] = [wqk.tile([128, 128], BF16, tag="wqk", name="wqb") for _ in range(DT)]
            for t in range(DT):
                nc.sync.dma_start(out=wqb_sb[p][t], in_=wqp[p, t * 128:(t + 1) * 128, :])

        kt_sb = [None] * NP   # [128, 2048] per pair (2x64 dh rows x keys)
        qt_sb = [None] * NP   # [128, 1024]

        def emit_proj_chunk(p, ci, pool=None):
            """One K/Q projection chunk for pair p: 8 MMs + 1 drain.
            ci 0-3: K chunks (half, kc); ci 4-5: Q chunks (qc)."""
            if ci == 0:
                kt_sb[p] = ktp.tile([128, 2048], BF16, tag="kt", name="kt")
            if ci == 4:
                qt_sb[p] = qtp.tile([128, 1024], BF16, tag="qt", name="qt")
            psk = (pool or prj).tile([128, 512], F32, tag="pvz" if pool else "prj",
                                     name="psk")
            if ci < 4:
                half, kc = ci // 2, ci % 2
                for t in range(DT):
                    nc.tensor.matmul(psk, wkb_sb[p][t],
                                     tokT_sb[t * 2 + half][:, kc * 512:(kc + 1) * 512],
                                     start=(t == 0), stop=(t == DT - 1))
                col = half * 1024 + kc * 512
                nc.vector.tensor_copy(kt_sb[p][:, col:col + 512], psk)
            else:
                qc = ci - 4
                for t in range(DT):
                    nc.tensor.matmul(psk, wqb_sb[p][t],
                                     tokTq_sb[t][:, qc * 512:(qc + 1) * 512],
                                     start=(t == 0), stop=(t == DT - 1))
                nc.vector.tensor_copy(qt_sb[p][:, qc * 512:(qc + 1) * 512], psk)

        # ---- attention per pair ----
        aoT2 = [None] * NP    # [128, 1024] bf16: rows 0-63 head 2p, 64-127 head 2p+1
        zrxi_sb = [None] * NP  # [128, 16] f32: 1/Z reshaped wide

        def drain_pair(p, pv, zb):
            """Fast pv/zb release at pair boundary: raw copies + Z reshape.
            zb rows {0,32,64,96} -> zrx [128, 16] (cols 4r..4r+3) so ONE DVE
            reciprocal covers all 2048 denominators at full lane parallelism
            (a [1,512] reciprocal costs 3.4us; [128,16] costs ~0.14us)."""
            aoT2[p] = aop.tile([128, 1024], BF16, tag="aoT2", name="aoT2")
            for qh in range(2):
                nc.scalar.copy(aoT2[p][:, qh * 512:(qh + 1) * 512], pv[qh])
            zsb = zsp.tile([128, 512], F32, tag="zsb", name="zsb")
            nc.vector.tensor_copy(zsb, zb)
            zrx = zsp.tile([128, 16], F32, tag="zrx", name="zrx")
            for r in range(4):
                nc.sync.dma_start(out=zrx[:, 4 * r:4 * r + 4],
                                  in_=zsb[32 * r:32 * r + 1, :])
            zrxi_sb[p] = zsp.tile([128, 16], F32, tag="zrxi", name="zrxi")
            nc.vector.reciprocal(zrxi_sb[p], zrx)

        def norm_step(p, qh):
            """Deferred normalization: scale aoT2[p] q-half by 1/Z (per head).
            zi2 = [1/Z(hi0) | 1/Z(hi1)] broadcast to all 128 partitions; each
            hi-half of aoT2 multiplies against its half at a MATCHING base
            partition (SBUF TT requires equal input base partitions)."""
            qsl = slice(qh * 512, (qh + 1) * 512)
            zi2 = zip_.tile([1, 1024], BF16, tag="zi2", name="zi2")
            for hi in range(2):
                r = 2 * hi + qh
                nc.gpsimd.dma_start(out=zi2[:, hi * 512:(hi + 1) * 512],
                                    in_=zrxi_sb[p][:, 4 * r:4 * r + 4])
            bc = bcp.tile([128, 1024], BF16, tag="bc", name="bc")
            nc.gpsimd.partition_broadcast(bc, zi2)
            for hi in range(2):
                hsl = slice(hi * 64, (hi + 1) * 64)
                nc.vector.tensor_mul(aoT2[p][hsl, qsl], aoT2[p][hsl, qsl],
                                     bc[hsl, hi * 512:(hi + 1) * 512])

        for p in range(NP):
            if p == 0:
                prefetch_w(0)
                prefetch_w(1)
                load_rf()
                for ci in range(6):
                    emit_proj_chunk(0, ci, pool=pvz)
            pv = [pvz.tile([128, 512], F32, tag="pvz", name="pv") for _ in range(2)]
            zb = pvz.tile([128, 512], F32, tag="pvz", name="zb")
            kt, qt = kt_sb[p], qt_sb[p]
            nci = 0  # next proj chunk to emit for pair p+1
            for it in range(32):
                kb, qh = it // 2, it % 2
                qsl = slice(qh * 512, (qh + 1) * 512)
                kcol = slice(kb * 128, (kb + 1) * 128)
                # S^T: row-tiled concurrent pair
                sps = sps_pool.tile([128, 1024], F32, tag="sps", name="sps")
                nc.tensor.matmul(sps[:, 0:512], kt[0:64, kcol], qt[0:64, qsl],
                                 start=True, stop=True)
                nc.tensor.matmul(sps[:, 512:1024], kt[64:128, kcol], qt[64:128, qsl],
                                 start=True, stop=True)
                # exp on ACT (PSUM -> SBUF bf16), then rfac muls on DVE (2x mode)
                e = expp.tile([128, 1024], BF16, tag="e", name="e")
                nc.scalar.activation(e, sps, mybir.ActivationFunctionType.Exp,
                                     scale=SCALE)
                rf = rfT_sb[kb][:, qsl]
                nc.vector.tensor_mul(e[:, 0:512], e[:, 0:512], rf)
                nc.vector.tensor_mul(e[:, 512:1024], e[:, 512:1024], rf)
                # PV col-tiled pair: A -> partitions 0-63, B -> 64-127
                nc.tensor.matmul(pv[qh][0:64, :], va_sb[kb][:, (2 * p) * 64:(2 * p) * 64 + 64],
                                 e[:, 0:512], start=(kb == 0), stop=(kb == KB - 1))
                nc.tensor.matmul(pv[qh][64:128, :], va_sb[kb][:, (2 * p + 1) * 64:(2 * p + 1) * 64 + 64],
                                 e[:, 512:1024], start=(kb == 0), stop=(kb == KB - 1))
                # Z: M=1 ones-matmuls, col strips qh and 2+qh
                nc.tensor.matmul(zb[32 * qh:32 * qh + 1, :], ones_t[:, 0:1],
                                 e[:, 0:512], start=(kb == 0), stop=(kb == KB - 1),
                                 tile_position=(0, 32 * qh))
                nc.tensor.matmul(zb[64 + 32 * qh:64 + 32 * qh + 1, :], ones_t[:, 0:1],
                                 e[:, 512:1024], start=(kb == 0), stop=(kb == KB - 1),
                                 tile_position=(0, 64 + 32 * qh))
                # interleave: next pair's K/Q proj chunks as PE filler
                if p + 1 < NP and it in (2, 6, 10, 14, 18, 22):
                    emit_proj_chunk(p + 1, nci)
                    nci += 1
                # deferred normalization of the previous pair, off critical path
                if p >= 1 and it == 8:
                    norm_step(p - 1, 0)
                if p >= 1 and it == 24:
                    norm_step(p - 1, 1)
                # deferred normalization of the previous pair, off critical path
            if p + 2 < NP:
                prefetch_w(p + 2)
            drain_pair(p, pv, zb)
        norm_step(NP - 1, 0)
        norm_step(NP - 1, 1)


        # ---- out projection + residual + layernorm ----
        wph_sb = []
        for hp in range(NP):
            s = big.tile([128, 1024], BF16, tag="big")
            nc.sync.dma_start(out=s, in_=wph2[hp])
            wph_sb.append(s)
        resid_sb = []
        for rb in range(8):
            s = big.tile([128, 1024], BF16, tag="big")
            nc.sync.dma_start(out=s, in_=resid[rb * 128:(rb + 1) * 128, :])
            resid_sb.append(s)

        for rb in range(8):
            rsl = slice(rb * 128, (rb + 1) * 128)
            psy = sps_pool.tile([128, 1024], F32, tag="sps", name="psy")
            for hp in range(NP):
                for nch in range(2):
                    nsl = slice(nch * 512, (nch + 1) * 512)
                    nc.tensor.matmul(psy[:, nsl], aoT2[hp][:, rsl],
                                     wph_sb[hp][:, nsl],
                                     start=(hp == 0), stop=(hp == NP - 1))
            x = xp.tile([128, 1024], F32, tag="x")
            nc.vector.tensor_add(x, psy, resid_sb[rb])
            st = smp.tile([128, 2, 6], F32, tag="st", name="st")
            for s2 in range(2):
                nc.vector.bn_stats(st[:, s2, :], x[:, s2 * 512:(s2 + 1) * 512])
            mv = smp.tile([128, 2], F32, tag="mv", name="mv")
            nc.vector.bn_aggr(mv, st)
            rstd = smp.tile([128, 1], F32, tag="rstd", name="rstd")
            nc.scalar.activation(rstd, mv[:, 1:2],
                                 mybir.ActivationFunctionType.Sqrt, bias=eps_t)
            nc.vector.reciprocal(rstd, rstd)
            nmr = smp.tile([128, 1], F32, tag="nmr", name="nmr")
            nc.vector.tensor_mul(nmr, mv[:, 0:1], rstd)
            nc.vector.tensor_scalar_mul(nmr, nmr, -1.0)
            nc.vector.tensor_scalar(x, x, nmr, rstd,
                                    mybir.AluOpType.add, mybir.AluOpType.mult)
            if not skip_affine:
                nc.vector.tensor_mul(x, x, gamma_b)
                nc.vector.tensor_add(x, x, lnb_b)
            nc.sync.dma_start(out=out[rsl, :], in_=x)


_CACHE = {}


def _build(skip_affine):
    key = bool(skip_affine)
    if key in _CACHE:
        return _CACHE[key]
    nc = bacc.Bacc("TRN2", target_bir_lowering=False, debug=False,
                   num_devices=NCORE)
    aps = (
        nc.dram_tensor("tokT", [D, L], BF16, kind="ExternalInput").ap(),
        nc.dram_tensor("tokTq", [D, LQ], BF16, kind="ExternalInput").ap(),
        nc.dram_tensor("wqp", [NP, D, 128], BF16, kind="ExternalInput").ap(),
        nc.dram_tensor("wkp", [NP, D, 128], BF16, kind="ExternalInput").ap(),
        nc.dram_tensor("wv", [D, D], BF16, kind="ExternalInput").ap(),
        nc.dram_tensor("wph2", [NP, 128, D], BF16, kind="ExternalInput").ap(),
        nc.dram_tensor("rfT", [L, LQ], BF16, kind="ExternalInput").ap(),
        nc.dram_tensor("resid", [LQ, D], BF16, kind="ExternalInput").ap(),
        nc.dram_tensor("gamma", [D], BF16, kind="ExternalInput").ap(),
        nc.dram_tensor("lnb", [D], BF16, kind="ExternalInput").ap(),
        nc.dram_tensor("out", [LQ, D], F32, kind="ExternalOutput").ap(),
    )
    with tile.TileContext(nc) as tc:
        _emit(tc, aps, skip_affine)
    nc.compile()
    _CACHE[key] = nc
    return nc


def kernel(tokens, R, Wq, Wk, Wv, Wp, beta, gamma, bias, _spmd_kwargs=None):
    tokens = np.asarray(tokens, dtype=np.float32)
    R = np.asarray(R, dtype=np.float32)
    skip_affine = bool(np.all(gamma == 1.0) and np.all(bias == 0.0))
    nc = _build(skip_affine)

    rfac = np.maximum(R, EPS_R).astype(np.float64) ** float(beta[0])
    rfacT = np.ascontiguousarray(rfac.T.astype(NPBF16))
    wqT = np.ascontiguousarray(Wq.T.astype(NPBF16))
    wkT = np.ascontiguousarray(Wk.T.astype(NPBF16))
    wvT = np.ascontiguousarray(Wv.T.astype(NPBF16))
    wpT = np.ascontiguousarray(Wp.T.astype(NPBF16))
    wqp = np.ascontiguousarray(wqT.reshape(D, NP, 128).transpose(1, 0, 2))
    wkp = np.ascontiguousarray(wkT.reshape(D, NP, 128).transpose(1, 0, 2))
    wph2 = np.ascontiguousarray(wpT.reshape(NP, 128, D))
    gamma_b = gamma.astype(NPBF16)
    lnb_b = bias.astype(NPBF16)

    in_maps = []
    for c in range(NCORE):
        b, q0 = c // 2, (c % 2) * LQ
        tokT = np.ascontiguousarray(tokens[b].T.astype(NPBF16))
        in_maps.append({
            "tokT": tokT,
            "tokTq": np.ascontiguousarray(tokT[:, q0:q0 + LQ]),
            "wqp": wqp, "wkp": wkp, "wv": wvT, "wph2": wph2,
            "rfT": np.ascontiguousarray(rfacT[:, q0:q0 + LQ]),
            "resid": np.ascontiguousarray(tokens[b, q0:q0 + LQ, :].astype(NPBF16)),
            "gamma": gamma_b, "lnb": lnb_b,
        })

    res = run_bass_kernel_spmd(nc, in_maps, core_ids=list(range(NCORE)),
                               **(_spmd_kwargs or {}))
    out = np.empty((B, L, D), dtype=np.float32)
    for c in range(NCORE):
        b, q0 = c // 2, (c % 2) * LQ
        out[b, q0:q0 + LQ, :] = res.results[c]["out"]
    if _spmd_kwargs:
        kernel._last_result = res
    return out


# revision 25
# speedup vs baseline: 1.3418x; 1.1918x over previous
"""CrossSeqAttentionLayer on 8 TRN2 NeuronCores.

Sharding: query-row split (no collectives). Core c handles batch c//2,
query rows (c%2)*1024 .. +1024, all 16 heads. Each core computes K/V for
its batch's full 2048 keys, attention for its 1024 query rows, the output
projection and layernorm for those rows, and writes [1024, 1024] f32.

Device-side structure (v2 — engine-balanced, PE-dense):
  - S^T per head-pair via ROW-TILED concurrent matmuls (K=64 in rows 0-63
    and 64-127 -> 2 MMs per 512-cycle slot).
  - One exp per [128 keys, 1024] PSUM tile (hi0|hi1) on ACT, then one DVE
    mul with rfac broadcast over the two head halves.
  - PV col-tiled: head A -> PSUM partitions 0-63, head B -> 64-127 of the
    same bank (concurrent M=64 matmuls). Softmax denominators via M=1
    ones-matmuls into strips of one shared Z bank (also col-concurrent).
  - K/Q projections of the next pair are interleaved into the attention
    stream as PE filler; V projection runs up front.
  - Out-projection contracts head PAIRS (K=128) with stationary reuse.
  - PSUM: 2x sps[128,1024] (4 banks) + 2x pv[128,512] + 1x Z + 1x proj = 8.
"""

import numpy as np
import ml_dtypes

import concourse.bass as bass
import concourse.mybir as mybir
import concourse.tile as tile
from concourse import bacc
from concourse.bass_utils import run_bass_kernel_spmd

BF16 = mybir.dt.bfloat16
F32 = mybir.dt.float32
NPBF16 = ml_dtypes.bfloat16

B, L, D = 4, 2048, 1024
H, DH = 16, 64
NCORE = 8
LQ = L // 2            # query rows per core
SCALE = DH ** -0.5
EPS_R = 1e-8
EPS_LN = 1e-5
NP = 8                 # head pairs
KB = L // 128          # key blocks of 128
DT = D // 128          # contraction tiles of 128


def _emit(tc, aps, skip_affine):
    nc = tc.nc
    (tokT, tokTq, wqp, wkp, wv, wph2, rfT, resid, gamma, lnb, out) = aps

    import contextlib
    with contextlib.ExitStack() as ctx:
        # SBUF pools
        big = ctx.enter_context(tc.tile_pool(name="big", bufs=64))   # resident [128,1024]-ish bf16
        ktp = ctx.enter_context(tc.tile_pool(name="ktp", bufs=2))    # [128,2048] kt per pair
        qtp = ctx.enter_context(tc.tile_pool(name="qtp", bufs=2))    # [128,1024] qt per pair
        wqk = ctx.enter_context(tc.tile_pool(name="wqk", bufs=32))   # [128,128] weight blocks
        expp = ctx.enter_context(tc.tile_pool(name="expp", bufs=3))  # e tiles [128,1024] bf16
        aop = ctx.enter_context(tc.tile_pool(name="aop", bufs=8))    # aoT2 [128,1024] bf16
        bcp = ctx.enter_context(tc.tile_pool(name="bcp", bufs=2))    # broadcast [128,1024] bf16
        zip_ = ctx.enter_context(tc.tile_pool(name="zip", bufs=4))   # zi2 [1,1024] bf16
        zsp = ctx.enter_context(tc.tile_pool(name="zsp", bufs=2))    # zsb [128,512] bf16
        smp = ctx.enter_context(tc.tile_pool(name="smp", bufs=8))   # small tiles
        xp = ctx.enter_context(tc.tile_pool(name="xp", bufs=2))      # x [128,1024] f32
        gbp = ctx.enter_context(tc.tile_pool(name="gbp", bufs=2))
        # PSUM pools — exactly 8 banks total
        sps_pool = ctx.enter_context(
            tc.tile_pool(name="sps", bufs=2, space=bass.MemorySpace.PSUM))   # [128,1024] -> 2 banks each
        pvz = ctx.enter_context(
            tc.tile_pool(name="pvz", bufs=3, space=bass.MemorySpace.PSUM))   # [128,512] pv0, pv1, zb
        prj = ctx.enter_context(
            tc.tile_pool(name="prj", bufs=1, space=bass.MemorySpace.PSUM))   # [128,512] proj chunks

        # ---- resident loads ----
        tokT_sb = []  # 16 tiles [128, 1024]: index t*2+half
        for t in range(DT):
            for half in range(2):
                s = big.tile([128, 1024], BF16, tag="big")
                nc.sync.dma_start(out=s, in_=tokT[t * 128:(t + 1) * 128,
                                               half * 1024:(half + 1) * 1024])
                tokT_sb.append(s)
        wv_sb = []
        for t in range(DT):
            s = big.tile([128, 1024], BF16, tag="big")
            nc.sync.dma_start(out=s, in_=wv[t * 128:(t + 1) * 128, :])
            wv_sb.append(s)
        tokTq_sb = []
        for t in range(DT):
            s = big.tile([128, 1024], BF16, tag="big")
            nc.sync.dma_start(out=s, in_=tokTq[t * 128:(t + 1) * 128, :])
            tokTq_sb.append(s)

        ones_t = smp.tile([128, 1], BF16, tag="ones")
        nc.vector.memset(ones_t, 1.0)
        eps_t = smp.tile([128, 1], F32, tag="eps")
        nc.vector.memset(eps_t, EPS_LN)
        if not skip_affine:
            gamma_b = gbp.tile([128, 1024], BF16)
            nc.gpsimd.dma_start(out=gamma_b, in_=gamma.partition_broadcast(128))
            lnb_b = gbp.tile([128, 1024], BF16)
            nc.gpsimd.dma_start(out=lnb_b, in_=lnb.partition_broadcast(128))

        # ---- V projection: va[kb] = V rows for 128 keys x all 16 heads ----
        va_sb = []
        for kb in range(KB):
            ps = sps_pool.tile([128, 1024], F32, tag="sps", name="psv")
            for vc in range(2):
                for t in range(DT):
                    lhs = tokT_sb[t * 2 + kb // 8][:, (kb % 8) * 128:(kb % 8 + 1) * 128]
                    nc.tensor.matmul(ps[:, vc * 512:(vc + 1) * 512], lhs,
                                     wv_sb[t][:, vc * 512:(vc + 1) * 512],
                                     start=(t == 0), stop=(t == DT - 1))
            va = big.tile([128, 1024], BF16, tag="big")
            nc.vector.tensor_copy(va, ps)
            va_sb.append(va)

        # ---- weight-block DMA prefetch for K/Q of pair p ----
        wkb_sb = [None] * NP
        wqb_sb = [None] * NP

        rfT_sb = [None] * KB

        def load_rf():
            for kb in range(KB):
                s = big.tile([128, 1024], BF16, tag="big")
                nc.sync.dma_start(out=s, in_=rfT[kb * 128:(kb + 1) * 128, :])
                rfT_sb[kb] = s

        def prefetch_w(p):
            wkb_sb[p] = [wqk.tile([128, 128], BF16, tag="wqk", name="wkb") for _ in range(DT)]
            for t in range(DT):
                nc.sync.dma_start(out=wkb_sb[p][t], in_=wkp[p, t * 128:(t + 1) * 128, :])
            wqb_sb[p] = [wqk.tile([128, 128], BF16, tag="wqk", name="wqb") for _ in range(DT)]
            for t in range(DT):
                nc.sync.dma_start(out=wqb_sb[p][t], in_=wqp[p, t * 128:(t + 1) * 128, :])

        kt_sb = [None] * NP   # [128, 2048] per pair (2x64 dh rows x keys)
        qt_sb = [None] * NP   # [128, 1024]

        def emit_proj_chunk(p, ci, pool=None):
            """One K/Q projection chunk for pair p: 8 MMs + 1 drain.
            ci 0-3: K chunks (half, kc); ci 4-5: Q chunks (qc)."""
            if ci == 0:
                kt_sb[p] = ktp.tile([128, 2048], BF16, tag="kt", name="kt")
            if ci == 4:
                qt_sb[p] = qtp.tile([128, 1024], BF16, tag="qt", name="qt")
            psk = (pool or prj).tile([128, 512], F32, tag="pvz" if pool else "prj",
                                     name="psk")
            if ci < 4:
                half, kc = ci // 2, ci % 2
                for t in range(DT):
                    nc.tensor.matmul(psk, wkb_sb[p][t],
                                     tokT_sb[t * 2 + half][:, kc * 512:(kc + 1) * 512],
                                     start=(t == 0), stop=(t == DT - 1))
                col = half * 1024 + kc * 512
                nc.vector.tensor_copy(kt_sb[p][:, col:col + 512], psk)
            else:
                qc = ci - 4
                for t in range(DT):
                    nc.tensor.matmul(psk, wqb_sb[p][t],
                                     tokTq_sb[t][:, qc * 512:(qc + 1) * 512],
                                     start=(t == 0), stop=(t == DT - 1))
                nc.vector.tensor_copy(qt_sb[p][:, qc * 512:(qc + 1) * 512], psk)

        # ---- attention per pair ----
        aoT2 = [None] * NP    # [128, 1024] bf16: rows 0-63 head 2p, 64-127 head 2p+1
        zrxi_sb = [None] * NP  # [128, 16] f32: 1/Z reshaped wide

        def drain_pair(p, pv, zb):
            """Fast pv/zb release at pair boundary: raw copies + Z reshape.
            zb rows {0,32,64,96} -> zrx [128, 16] (cols 4r..4r+3) so ONE DVE
            reciprocal covers all 2048 denominators at full lane parallelism
            (a [1,512] reciprocal costs 3.4us; [128,16] costs ~0.14us)."""
            aoT2[p] = aop.tile([128, 1024], BF16, tag="aoT2", name="aoT2")
            for qh in range(2):
                nc.scalar.copy(aoT2[p][:, qh * 512:(qh + 1) * 512], pv[qh])
            zsb = zsp.tile([128, 512], F32, tag="zsb", name="zsb")
            nc.vector.tensor_copy(zsb, zb)
            zrx = zsp.tile([128, 16], F32, tag="zrx", name="zrx")
            for r in range(4):
                nc.sync.dma_start(out=zrx[:, 4 * r:4 * r + 4],
                                  in_=zsb[32 * r:32 * r + 1, :])
            zrxi_sb[p] = zsp.tile([128, 16], F32, tag="zrxi", name="zrxi")
            nc.vector.reciprocal(zrxi_sb[p], zrx)

        def norm_step(p, qh):
            """Deferred normalization: scale aoT2[p] q-half by 1/Z (per head).
            zi2 = [1/Z(hi0) | 1/Z(hi1)] broadcast to all 128 partitions; each
            hi-half of aoT2 multiplies against its half at a MATCHING base
            partition (SBUF TT requires equal input base partitions)."""
            qsl = slice(qh * 512, (qh + 1) * 512)
            zi2 = zip_.tile([1, 1024], BF16, tag="zi2", name="zi2")
            for hi in range(2):
                r = 2 * hi + qh
                nc.gpsimd.dma_start(out=zi2[:, hi * 512:(hi + 1) * 512],
                                    in_=zrxi_sb[p][:, 4 * r:4 * r + 4])
            bc = bcp.tile([128, 1024], BF16, tag="bc", name="bc")
            nc.gpsimd.partition_broadcast(bc, zi2)
            for hi in range(2):
                hsl = slice(hi * 64, (hi + 1) * 64)
                nc.vector.tensor_mul(aoT2[p][hsl, qsl], aoT2[p][hsl, qsl],
                                     bc[hsl, hi * 512:(hi + 1) * 512])

        for p in range(NP):
            if p == 0:
                prefetch_w(0)
                prefetch_w(1)
                load_rf()
                for ci in range(6):
                    emit_proj_chunk(0, ci, pool=pvz)
            pv = [pvz.tile([128, 512], F32, tag="pvz", name="pv") for _ in range(2)]
            zb = pvz.tile([128, 512], F32, tag="pvz", name="zb")
            kt, qt = kt_sb[p], qt_sb[p]
            nci = 0  # next proj chunk to emit for pair p+1
            for it in range(32):
                kb, qh = it // 2, it % 2
                qsl = slice(qh * 512, (qh + 1) * 512)
                kcol = slice(kb * 128, (kb + 1) * 128)
                # S^T: row-tiled concurrent pair
                sps = sps_pool.tile([128, 1024], F32, tag="sps", name="sps")
                nc.tensor.matmul(sps[:, 0:512], kt[0:64, kcol], qt[0:64, qsl],
                                 start=True, stop=True)
                nc.tensor.matmul(sps[:, 512:1024], kt[64:128, kcol], qt[64:128, qsl],
                                 start=True, stop=True)
                # exp on ACT (PSUM -> SBUF bf16), then rfac muls on DVE (2x mode)
                e = expp.tile([128, 1024], BF16, tag="e", name="e")
                nc.scalar.activation(e, sps, mybir.ActivationFunctionType.Exp,
                                     scale=SCALE)
                rf = rfT_sb[kb][:, qsl]
                # hi1 half FIRST: the A-side matmuls below then wait on the
                # later-finishing hi0 mul, so A/B matmul pairs stay concurrent
                nc.vector.tensor_mul(e[:, 512:1024], e[:, 512:1024], rf)
                nc.vector.tensor_mul(e[:, 0:512], e[:, 0:512], rf)
                # PV col-tiled pair: A -> partitions 0-63, B -> 64-127
                nc.tensor.matmul(pv[qh][0:64, :], va_sb[kb][:, (2 * p) * 64:(2 * p) * 64 + 64],
                                 e[:, 0:512], start=(kb == 0), stop=(kb == KB - 1))
                nc.tensor.matmul(pv[qh][64:128, :], va_sb[kb][:, (2 * p + 1) * 64:(2 * p + 1) * 64 + 64],
                                 e[:, 512:1024], start=(kb == 0), stop=(kb == KB - 1))
                # Z: M=1 ones-matmuls, col strips qh and 2+qh
                nc.tensor.matmul(zb[32 * qh:32 * qh + 1, :], ones_t[:, 0:1],
                                 e[:, 0:512], start=(kb == 0), stop=(kb == KB - 1),
                                 tile_position=(0, 32 * qh))
                nc.tensor.matmul(zb[64 + 32 * qh:64 + 32 * qh + 1, :], ones_t[:, 0:1],
                                 e[:, 512:1024], start=(kb == 0), stop=(kb == KB - 1),
                                 tile_position=(0, 64 + 32 * qh))
                # interleave: next pair's K/Q proj chunks as PE filler
                if p + 1 < NP and it in (2, 6, 10, 14, 18, 22):
                    emit_proj_chunk(p + 1, nci)
                    nci += 1
                # deferred normalization of the previous pair, off critical path
                if p >= 1 and it == 8:
                    norm_step(p - 1, 0)
                if p >= 1 and it == 24:
                    norm_step(p - 1, 1)
                # deferred normalization of the previous pair, off critical path
            if p + 2 < NP:
                prefetch_w(p + 2)
            drain_pair(p, pv, zb)
        norm_step(NP - 1, 0)
        norm_step(NP - 1, 1)


        # ---- out projection + residual + layernorm ----
        wph_sb = []
        for hp in range(NP):
            s = big.tile([128, 1024], BF16, tag="big")
            nc.sync.dma_start(out=s, in_=wph2[hp])
            wph_sb.append(s)
        resid_sb = []
        for rb in range(8):
            s = big.tile([128, 1024], BF16, tag="big")
            nc.sync.dma_start(out=s, in_=resid[rb * 128:(rb + 1) * 128, :])
            resid_sb.append(s)

        for rb in range(8):
            rsl = slice(rb * 128, (rb + 1) * 128)
            psy = sps_pool.tile([128, 1024], F32, tag="sps", name="psy")
            for hp in range(NP):
                for nch in range(2):
                    nsl = slice(nch * 512, (nch + 1) * 512)
                    nc.tensor.matmul(psy[:, nsl], aoT2[hp][:, rsl],
                                     wph_sb[hp][:, nsl],
                                     start=(hp == 0), stop=(hp == NP - 1))
            x = xp.tile([128, 1024], F32, tag="x")
            nc.vector.tensor_add(x, psy, resid_sb[rb])
            st = smp.tile([128, 2, 6], F32, tag="st", name="st")
            for s2 in range(2):
                nc.vector.bn_stats(st[:, s2, :], x[:, s2 * 512:(s2 + 1) * 512])
            mv = smp.tile([128, 2], F32, tag="mv", name="mv")
            nc.vector.bn_aggr(mv, st)
            rstd = smp.tile([128, 1], F32, tag="rstd", name="rstd")
            nc.scalar.activation(rstd, mv[:, 1:2],
                                 mybir.ActivationFunctionType.Sqrt, bias=eps_t)
            nc.vector.reciprocal(rstd, rstd)
            nmr = smp.tile([128, 1], F32, tag="nmr", name="nmr")
            nc.vector.tensor_mul(nmr, mv[:, 0:1], rstd)
            nc.vector.tensor_scalar_mul(nmr, nmr, -1.0)
            nc.vector.tensor_scalar(x, x, nmr, rstd,
                                    mybir.AluOpType.add, mybir.AluOpType.mult)
            if not skip_affine:
                nc.vector.tensor_mul(x, x, gamma_b)
                nc.vector.tensor_add(x, x, lnb_b)
            nc.sync.dma_start(out=out[rsl, :], in_=x)


_CACHE = {}


def _build(skip_affine):
    key = bool(skip_affine)
    if key in _CACHE:
        return _CACHE[key]
    nc = bacc.Bacc("TRN2", target_bir_lowering=False, debug=False,
                   num_devices=NCORE)
    aps = (
        nc.dram_tensor("tokT", [D, L], BF16, kind="ExternalInput").ap(),
        nc.dram_tensor("tokTq", [D, LQ], BF16, kind="ExternalInput").ap(),
        nc.dram_tensor("wqp", [NP, D, 128], BF16, kind="ExternalInput").ap(),
        nc.dram_tensor("wkp", [NP, D, 128], BF16, kind="ExternalInput").ap(),
        nc.dram_tensor("wv", [D, D], BF16, kind="ExternalInput").ap(),
        nc.dram_tensor("wph2", [NP, 128, D], BF16, kind="ExternalInput").ap(),
        nc.dram_tensor("rfT", [L, LQ], BF16, kind="ExternalInput").ap(),
        nc.dram_tensor("resid", [LQ, D], BF16, kind="ExternalInput").ap(),
        nc.dram_tensor("gamma", [D], BF16, kind="ExternalInput").ap(),
        nc.dram_tensor("lnb", [D], BF16, kind="ExternalInput").ap(),
        nc.dram_tensor("out", [LQ, D], F32, kind="ExternalOutput").ap(),
    )
    with tile.TileContext(nc) as tc:
        _emit(tc, aps, skip_affine)
    nc.compile()
    _CACHE[key] = nc
    return nc


def kernel(tokens, R, Wq, Wk, Wv, Wp, beta, gamma, bias, _spmd_kwargs=None):
    tokens = np.asarray(tokens, dtype=np.float32)
    R = np.asarray(R, dtype=np.float32)
    skip_affine = bool(np.all(gamma == 1.0) and np.all(bias == 0.0))
    nc = _build(skip_affine)

    rfac = np.maximum(R, EPS_R).astype(np.float64) ** float(beta[0])
    rfacT = np.ascontiguousarray(rfac.T.astype(NPBF16))
    wqT = np.ascontiguousarray(Wq.T.astype(NPBF16))
    wkT = np.ascontiguousarray(Wk.T.astype(NPBF16))
    wvT = np.ascontiguousarray(Wv.T.astype(NPBF16))
    wpT = np.ascontiguousarray(Wp.T.astype(NPBF16))
    wqp = np.ascontiguousarray(wqT.reshape(D, NP, 128).transpose(1, 0, 2))
    wkp = np.ascontiguousarray(wkT.reshape(D, NP, 128).transpose(1, 0, 2))
    wph2 = np.ascontiguousarray(wpT.reshape(NP, 128, D))
    gamma_b = gamma.astype(NPBF16)
    lnb_b = bias.astype(NPBF16)

    in_maps = []
    for c in range(NCORE):
        b, q0 = c // 2, (c % 2) * LQ
        tokT = np.ascontiguousarray(tokens[b].T.astype(NPBF16))
        in_maps.append({
            "tokT": tokT,
            "tokTq": np.ascontiguousarray(tokT[:, q0:q0 + LQ]),
            "wqp": wqp, "wkp": wkp, "wv": wvT, "wph2": wph2,
            "rfT": np.ascontiguousarray(rfacT[:, q0:q0 + LQ]),
            "resid": np.ascontiguousarray(tokens[b, q0:q0 + LQ, :].astype(NPBF16)),
            "gamma": gamma_b, "lnb": lnb_b,
        })

    res = run_bass_kernel_spmd(nc, in_maps, core_ids=list(range(NCORE)),
                               **(_spmd_kwargs or {}))
    out = np.empty((B, L, D), dtype=np.float32)
    for c in range(NCORE):
        b, q0 = c // 2, (c % 2) * LQ
        out[b, q0:q0 + LQ, :] = res.results[c]["out"]
    if _spmd_kwargs:
        kernel._last_result = res
    return out
